# revision 5
# baseline (speedup 1.0000x reference)
"""Trainium2 Bass kernel for nn_DataEmbedding (linear embed + positional + GCN).

out[b,n,t,:] = x[b,n,t,:] @ W_lin + b_lin + pe[t,:] + gcn(emb_table)[n,:]

Sharding: graph-partitioned by destination node. Core k owns nodes
[625k, 625(k+1)) and produces the output shard out[:, 625k:625(k+1), :, :].
No collectives. Host does index/layout prep only (edge scatter into a dense
adjacency, padding, bf16 casts); all floating-point math runs on device.

GCN message passing is a dense matmul: the host scatters raw edge weights
into A[src, dst] (bf16, [5120 x 625] per core, ~0.6% dense), and the device
computes vp = A^T @ g with 40 accumulating 128-contraction matmuls per
125-node block, where g = D^-1/2 (emb @ W_gcn). Destination normalization
and bias fold into the per-block ve finalize. This replaces the per-edge
indirect-DMA gather + one-hot scatter (which was descriptor-bound).

Main output: per (block, batch) a [38 x 125] bf16 lhsT (x rows + ones rows
for pe/bias) hits a [38 x 3072] rhs; PSUM is evacuated with the ve add,
split between the Vector and Scalar engines, and written to DRAM in bf16.
"""

import numpy as np
import ml_dtypes

import concourse.bacc as bacc
import concourse.bass as bass
import concourse.mybir as mybir
from concourse.bass_utils import run_bass_kernel_spmd
from concourse.tile import TileContext

# problem constants (hardcoded per contract)
B, N, T, CIN, D, E = 8, 5000, 12, 3, 256, 160000
NCORES = 8
NPC = N // NCORES        # nodes per core = 625
BLK = 125                # destination nodes per PSUM block
NBLK = NPC // BLK        # blocks per core = 5
NT = (N + 127) // 128    # global 128-node source tiles = 40
NP = NT * 128            # padded source count = 5120
KX = 3 * T + 2           # main matmul contraction: (t,c) rows + pe + bias = 38
TP = T * D // 512        # 512-col tiles across (t,d) = 6
HALF = 3 * 512           # free elems per evacuation half = 1536

f32 = mybir.dt.float32
b16 = mybir.dt.bfloat16

bf = ml_dtypes.bfloat16


def _pe_table() -> np.ndarray:
    pos = np.arange(T, dtype=np.float32)[:, None]
    div = np.exp(np.arange(0, D, 2, dtype=np.float32) * (-np.log(10000.0) / D))
    pe = np.zeros((T, D), dtype=np.float32)
    pe[:, 0::2] = np.sin(pos * div)
    pe[:, 1::2] = np.cos(pos * div)
    return pe


def _prep(x, edge_index, weights, W_lin, b_lin):
    """Host-side sharding/layout prep: edge scatter, padding, bf16 casts."""
    ei = np.asarray(edge_index)
    row2 = np.concatenate([ei[0], np.arange(N)]).astype(np.int64)  # src
    col2 = np.concatenate([ei[1], np.arange(N)]).astype(np.int64)  # dst
    w2 = np.concatenate(
        [np.asarray(weights, dtype=np.float32), np.ones(N, dtype=np.float32)]
    )

    # dense adjacency A[src, dst] of raw weights (self-loops w=1 included);
    # duplicate (src,dst) edges accumulate, matching segment_sum semantics
    A = np.zeros((NP, N), dtype=np.float32)
    np.add.at(A, (row2, col2), w2)

    # padded per-node incoming-weight matrix for on-device degree = row-sum
    order = np.argsort(col2, kind="stable")
    col_s, w_s = col2[order], w2[order]
    starts = np.searchsorted(col_s, np.arange(N)).astype(np.int64)
    cnt = np.bincount(col2, minlength=N)
    L = int(max(8, ((cnt.max() + 7) // 8) * 8))
    wpad = np.zeros((NP, L), dtype=np.float32)
    offs = np.arange(len(col_s), dtype=np.int64) - starts[col_s]
    wpad[col_s, offs] = w_s
    wpad[N:, 0] = 1.0  # pad rows: deg=1 so dinv stays finite
    wpad_pm = np.ascontiguousarray(
        wpad.reshape(NT, 128, L).transpose(1, 0, 2).reshape(128, NT * L)
    )

    # main-matmul rhs [KX, T*D]: rows 3t+c carry W_lin[c] in the t-block of
    # columns, row 36 = positional encoding, row 37 = b_lin tiled
    pe = _pe_table()
    rhs38 = np.zeros((KX, T * D), dtype=np.float32)
    for t in range(T):
        for c in range(CIN):
            rhs38[3 * t + c, t * D : (t + 1) * D] = np.asarray(W_lin, np.float32)[c]
    rhs38[36] = pe.reshape(-1)
    rhs38[37] = np.tile(np.asarray(b_lin, dtype=np.float32), T)

    xa = np.asarray(x, dtype=np.float32)
    per_core = []
    for k in range(NCORES):
        # A tiles in matmul lhsT layout: [128 src-partition,
        # (blk*NT + j)*BLK + dst-local] bf16
        Ak = A[:, k * NPC : (k + 1) * NPC]
        A_sb = np.ascontiguousarray(
            Ak.reshape(NT, 128, NBLK, BLK)
            .transpose(1, 2, 0, 3)
            .reshape(128, NBLK * NT * BLK)
            .astype(bf)
        )
        # x in matmul-ready lhsT layout [KX, NBLK*B*BLK] bf16: rows are
        # (t,c) pairs then two ones-rows (pe, bias)
        xs = xa[:, k * NPC : (k + 1) * NPC].reshape(B, NBLK, BLK, T, CIN)
        x38 = np.ones((KX, NBLK, B, BLK), dtype=np.float32)
        x38[: 3 * T] = xs.transpose(3, 4, 1, 0, 2).reshape(3 * T, NBLK, B, BLK)
        per_core.append(
            {
                "A": A_sb,
                "x38": np.ascontiguousarray(x38.reshape(KX, NBLK * B * BLK)).astype(bf),
                "wpad_loc": np.ascontiguousarray(
                    wpad[k * NPC : (k + 1) * NPC]
                    .reshape(NBLK, BLK, L)
                    .transpose(1, 0, 2)
                    .reshape(BLK, NBLK * L)
                ),
            }
        )
    return per_core, wpad_pm, rhs38.astype(bf), L


_KERNEL_CACHE: dict = {}


def _build_kernel(L: int):
    if L in _KERNEL_CACHE:
        return _KERNEL_CACHE[L]

    nc = bacc.Bacc()
    x38_d = nc.declare_dram_parameter("x38", [KX, NBLK * B * BLK], b16, isOutput=False)
    A_d = nc.declare_dram_parameter("A", [128, NBLK * NT * BLK], b16, isOutput=False)
    wpad_d = nc.declare_dram_parameter("wpad", [128, NT * L], f32, isOutput=False)
    wploc_d = nc.declare_dram_parameter("wpad_loc", [BLK, NBLK * L], f32, isOutput=False)
    embT_d = nc.declare_dram_parameter("embT", [D, N], b16, isOutput=False)
    wg_d = nc.declare_dram_parameter("W_gcn", [D, D], b16, isOutput=False)
    bg_d = nc.declare_dram_parameter("b_gcn", [1, D], f32, isOutput=False)
    rhs38_d = nc.declare_dram_parameter("rhs38", [KX, T * D], b16, isOutput=False)
    out_d = nc.declare_dram_parameter("out", [B, NPC, T, D], b16, isOutput=True)

    with TileContext(nc) as tc:
        with tc.tile_pool(name="const", bufs=1) as cp:
            ones_row = cp.tile([1, BLK], f32)
            nc.vector.memset(ones_row[:], 1.0)

            w_all = cp.tile([128, NT * L], f32)
            wl_all = cp.tile([BLK, NBLK * L], f32)
            nc.scalar.dma_start(out=w_all[:], in_=wpad_d[:])
            nc.scalar.dma_start(out=wl_all[:], in_=wploc_d[:])

            wg0 = cp.tile([128, D], b16)
            wg1 = cp.tile([128, D], b16)
            nc.scalar.dma_start(out=wg0[:], in_=wg_d[0:128, :])
            nc.scalar.dma_start(out=wg1[:], in_=wg_d[128:256, :])
            bg_row = cp.tile([1, D], f32)
            nc.scalar.dma_start(out=bg_row[:], in_=bg_d[:])

            rhs38 = cp.tile([KX, T * D], b16)
            nc.scalar.dma_start(out=rhs38[:], in_=rhs38_d[:])
            x38 = cp.tile([KX, NBLK * B * BLK], b16)
            nc.scalar.dma_start(out=x38[:], in_=x38_d[:])

            # A loads ride the Sync HWDGE ring (5 SDMA engines) to keep the
            # 16-engine Scalar ring free for the output writes
            A_sb = []
            for blk in range(NBLK):
                a = cp.tile([128, NT * BLK], b16, tag=f"A{blk}")
                nc.sync.dma_start(
                    out=a[:], in_=A_d[:, blk * NT * BLK : (blk + 1) * NT * BLK]
                )
                A_sb.append(a)

            dinv_all = cp.tile([128, NT], f32)
            dinv_loc = cp.tile([BLK, NBLK], f32)
            g_all = cp.tile([128, NT * D], b16)
            b_rep = cp.tile([BLK, D], f32)

            # ---- phase A: degrees -> dinv (batched reduce + rsqrt) ----
            with (
                tc.tile_pool(name="pA", bufs=2) as pA,
                tc.tile_pool(name="ppA", bufs=2, space="PSUM") as ppA,
                tc.tile_pool(name="emb", bufs=1) as ep,
            ):
                eT0 = ep.tile([128, N], b16)
                eT1 = ep.tile([128, N], b16)
                nc.scalar.dma_start(out=eT0[:], in_=embT_d[0:128, :])
                nc.scalar.dma_start(out=eT1[:], in_=embT_d[128:256, :])

                dega = pA.tile([128, NT], f32, tag="dega")
                nc.vector.reduce_sum(
                    out=dega[:],
                    in_=w_all[:].rearrange("p (j l) -> p j l", l=L),
                    axis=mybir.AxisListType.X,
                )
                reca = pA.tile([128, NT], f32, tag="reca")
                nc.vector.reciprocal(reca[:], dega[:])
                nc.scalar.sqrt(dinv_all[:], reca[:])

                degl = pA.tile([BLK, NBLK], f32, tag="degl")
                nc.vector.reduce_sum(
                    out=degl[:],
                    in_=wl_all[:].rearrange("p (j l) -> p j l", l=L),
                    axis=mybir.AxisListType.X,
                )
                recl = pA.tile([BLK, NBLK], f32, tag="recl")
                nc.vector.reciprocal(recl[:], degl[:])
                nc.scalar.sqrt(dinv_loc[:], recl[:])

                # ---- phase B: g = dinv * (emb @ W_gcn), bf16 in SBUF ----
                nc.vector.memset(g_all[:, (NT - 1) * D :], 0.0)
                for j in range(NT):
                    cols = min(128, N - j * 128)
                    hg = ppA.tile([128, D], f32, space="PSUM", tag="hg")
                    nc.tensor.matmul(
                        hg[:cols, :],
                        lhsT=eT0[:, j * 128 : j * 128 + cols],
                        rhs=wg0[:],
                        start=True,
                        stop=False,
                    )
                    nc.tensor.matmul(
                        hg[:cols, :],
                        lhsT=eT1[:, j * 128 : j * 128 + cols],
                        rhs=wg1[:],
                        start=False,
                        stop=True,
                    )
                    nc.scalar.mul(
                        g_all[:cols, j * D : (j + 1) * D],
                        hg[:cols, :],
                        dinv_all[:cols, j : j + 1],
                    )
                # b_rep = ones(125,1) @ b_gcn(1,256)
                br = ppA.tile([BLK, D], f32, space="PSUM", tag="hg")
                nc.tensor.matmul(
                    br[:], lhsT=ones_row[0:1, :], rhs=bg_row[0:1, :],
                    start=True, stop=True,
                )
                nc.vector.tensor_copy(b_rep[:], br[:])

            # ---- phase C: per block, dense-A GCN matmul then main output ----
            with (
                tc.tile_pool(name="vef", bufs=2) as vef,
                tc.tile_pool(name="veb", bufs=2) as veb,
                tc.tile_pool(name="vps", bufs=2, space="PSUM") as vps,
                tc.tile_pool(name="mps", bufs=2, space="PSUM") as mps,
                tc.tile_pool(name="outp", bufs=3) as outp,
            ):
                for blk in range(NBLK):
                    # vp[dst, :] = sum_src A[src, dst] * g[src, :]
                    vp = vps.tile([BLK, D], f32, space="PSUM", tag="vp")
                    for j in range(NT):
                        nc.tensor.matmul(
                            vp[:],
                            lhsT=A_sb[blk][:, j * BLK : (j + 1) * BLK],
                            rhs=g_all[:, j * D : (j + 1) * D],
                            start=(j == 0),
                            stop=(j == NT - 1),
                        )
                    ve = vef.tile([BLK, D], f32, tag="ve")
                    nc.scalar.mul(ve[:], vp[:], dinv_loc[:, blk : blk + 1])
                    # veps = (ve + b_gcn) tiled over the 6 t-slots of a half
                    veps = veb.tile([BLK, HALF], b16, tag="veps")
                    nc.vector.tensor_tensor(
                        out=veps[:].rearrange("p (t d) -> p t d", d=D),
                        in0=ve[:].rearrange("p d -> p () d").to_broadcast(
                            [BLK, HALF // D, D]
                        ),
                        in1=b_rep[:].rearrange("p d -> p () d").to_broadcast(
                            [BLK, HALF // D, D]
                        ),
                        op=mybir.AluOpType.add,
                    )

                    for b in range(B):
                        lhsT = x38[:, (blk * B + b) * BLK : (blk * B + b + 1) * BLK]
                        osb = outp.tile([BLK, T * D], b16, tag="osb")
                        for half in range(2):
                            mp = mps.tile([BLK, HALF], f32, space="PSUM", tag="mp")
                            for i in range(3):
                                tp = half * 3 + i
                                nc.tensor.matmul(
                                    mp[:, i * 512 : (i + 1) * 512],
                                    lhsT=lhsT,
                                    rhs=rhs38[:, tp * 512 : (tp + 1) * 512],
                                    start=True,
                                    stop=True,
                                )
                            dst = osb[:, half * HALF : (half + 1) * HALF]
                            h = (blk * B + b) * 2 + half
                            if h % 4 == 0:
                                # route a: DVE adds ve while evacuating PSUM
                                nc.vector.tensor_tensor(
                                    out=dst.rearrange("p (t d) -> p t d", d=D),
                                    in0=mp[:].rearrange("p (t d) -> p t d", d=D),
                                    in1=veps[:].rearrange("p (t d) -> p t d", d=D),
                                    op=mybir.AluOpType.add,
                                )
                            else:
                                # route b: ACT evacuates PSUM to bf16, DVE
                                # adds ve in-place at 2x (16-bit) rate
                                nc.scalar.copy(dst, mp[:])
                                nc.vector.tensor_tensor(
                                    out=dst,
                                    in0=dst,
                                    in1=veps[:],
                                    op=mybir.AluOpType.add,
                                )
                        # output writes on the Scalar HWDGE ring: its queue
                        # spans all 16 SDMA engines (Sync's spans only 5)
                        nc.scalar.dma_start(
                            out=out_d[b, blk * BLK : (blk + 1) * BLK].rearrange(
                                "p t d -> p (t d)"
                            ),
                            in_=osb[:],
                        )

    nc.finalize()  # run bacc passes (reg alloc, TRN2 sync-wait splitting)
    _KERNEL_CACHE[L] = nc
    return nc


LAST_RESULTS = None  # BassKernelResults of the most recent run (for profiling)


def kernel(x, x_mark, edge_index, weights, W_lin, b_lin, emb_table, W_gcn, b_gcn):
    global LAST_RESULTS
    per_core, wpad, rhs38, L = _prep(x, edge_index, weights, W_lin, b_lin)
    nc = _build_kernel(L)
    embT = np.ascontiguousarray(np.asarray(emb_table, dtype=np.float32).T.astype(bf))
    shared = {
        "wpad": wpad,
        "embT": embT,
        "W_gcn": np.asarray(W_gcn, dtype=np.float32).astype(bf),
        "b_gcn": np.asarray(b_gcn, dtype=np.float32).reshape(1, D),
        "rhs38": rhs38,
    }
    in_maps = [{**shared, **pc} for pc in per_core]
    res = run_bass_kernel_spmd(nc, in_maps, list(range(NCORES)))
    LAST_RESULTS = res
    shards = [np.asarray(res.results[k]["out"]) for k in range(NCORES)]
    return np.concatenate(shards, axis=1).astype(np.float32)


# revision 11
# speedup vs baseline: 1.4571x; 1.4571x over previous
"""Trainium2 Bass kernel for nn_DataEmbedding (linear embed + positional + GCN).

out[b,n,t,:] = x[b,n,t,:] @ W_lin + b_lin + pe[t,:] + gcn(emb_table)[n,:]

Sharding: graph-partitioned by destination node. Core k owns nodes
[625k, 625(k+1)) and produces the output shard out[:, 625k:625(k+1), :, :].
No collectives. Host does index/layout prep only (edge scatter into a dense
adjacency, padding, bf16 casts); all floating-point math runs on device.

GCN message passing is a dense matmul: the host scatters raw edge weights
into A[src, dst] (bf16, [5120 x 625] per core, ~0.6% dense), and the device
computes vp = A^T @ g with 40 accumulating 128-contraction matmuls per
125-node block, where g = D^-1/2 (emb @ W_gcn). Destination normalization
and bias fold into the per-block ve finalize. This replaces the per-edge
indirect-DMA gather + one-hot scatter (which was descriptor-bound).

Main output: per (block, batch) a [38 x 125] bf16 lhsT (x rows + ones rows
for pe/bias) hits a [38 x 3072] rhs; PSUM is evacuated with the ve add,
split between the Vector and Scalar engines, and written to DRAM in bf16.
"""

import numpy as np
import ml_dtypes

import concourse.bacc as bacc
import concourse.bass as bass
import concourse.mybir as mybir
from concourse.bass_utils import run_bass_kernel_spmd
from concourse.tile import TileContext

# problem constants (hardcoded per contract)
B, N, T, CIN, D, E = 8, 5000, 12, 3, 256, 160000
NCORES = 8
NPC = N // NCORES        # nodes per core = 625
BLK = 125                # destination nodes per PSUM block
NBLK = NPC // BLK        # blocks per core = 5
NT = (N + 127) // 128    # global 128-node source tiles = 40
NP = NT * 128            # padded source count = 5120
KX = 3 * T + 2           # main matmul contraction: (t,c) rows + pe + bias = 38
TP = T * D // 512        # 512-col tiles across (t,d) = 6
HALF = 3 * 512           # free elems per evacuation half = 1536

f32 = mybir.dt.float32
b16 = mybir.dt.bfloat16

bf = ml_dtypes.bfloat16


def _pe_table() -> np.ndarray:
    pos = np.arange(T, dtype=np.float32)[:, None]
    div = np.exp(np.arange(0, D, 2, dtype=np.float32) * (-np.log(10000.0) / D))
    pe = np.zeros((T, D), dtype=np.float32)
    pe[:, 0::2] = np.sin(pos * div)
    pe[:, 1::2] = np.cos(pos * div)
    return pe


def _prep(x, edge_index, weights, W_lin, b_lin):
    """Host-side sharding/layout prep: edge scatter, padding, bf16 casts."""
    ei = np.asarray(edge_index)
    row2 = np.concatenate([ei[0], np.arange(N)]).astype(np.int64)  # src
    col2 = np.concatenate([ei[1], np.arange(N)]).astype(np.int64)  # dst
    w2 = np.concatenate(
        [np.asarray(weights, dtype=np.float32), np.ones(N, dtype=np.float32)]
    )

    # dense adjacency A[src, dst] of raw weights (self-loops w=1 included);
    # duplicate (src,dst) edges accumulate, matching segment_sum semantics
    A = np.zeros((NP, N), dtype=np.float32)
    np.add.at(A, (row2, col2), w2)

    # padded per-node incoming-weight matrix for on-device degree = row-sum
    order = np.argsort(col2, kind="stable")
    col_s, w_s = col2[order], w2[order]
    starts = np.searchsorted(col_s, np.arange(N)).astype(np.int64)
    cnt = np.bincount(col2, minlength=N)
    L = int(max(8, ((cnt.max() + 7) // 8) * 8))
    wpad = np.zeros((NP, L), dtype=np.float32)
    offs = np.arange(len(col_s), dtype=np.int64) - starts[col_s]
    wpad[col_s, offs] = w_s
    wpad[N:, 0] = 1.0  # pad rows: deg=1 so dinv stays finite
    wpad_pm = np.ascontiguousarray(
        wpad.reshape(NT, 128, L).transpose(1, 0, 2).reshape(128, NT * L)
    )

    # main-matmul rhs [KX, T*D]: rows 3t+c carry W_lin[c] in the t-block of
    # columns, row 36 = positional encoding, row 37 = b_lin tiled
    pe = _pe_table()
    rhs38 = np.zeros((KX, T * D), dtype=np.float32)
    for t in range(T):
        for c in range(CIN):
            rhs38[3 * t + c, t * D : (t + 1) * D] = np.asarray(W_lin, np.float32)[c]
    rhs38[36] = pe.reshape(-1)
    rhs38[37] = np.tile(np.asarray(b_lin, dtype=np.float32), T)

    xa = np.asarray(x, dtype=np.float32)
    per_core = []
    for k in range(NCORES):
        # A tiles in matmul lhsT layout: [128 src-partition,
        # (blk*NT + j)*BLK + dst-local] bf16
        Ak = A[:, k * NPC : (k + 1) * NPC]
        A_sb = np.ascontiguousarray(
            Ak.reshape(NT, 128, NBLK, BLK)
            .transpose(1, 2, 0, 3)
            .reshape(128, NBLK * NT * BLK)
            .astype(bf)
        )
        # x in matmul-ready lhsT layout [KX, NBLK*B*BLK] bf16: rows are
        # (t,c) pairs then two ones-rows (pe, bias)
        xs = xa[:, k * NPC : (k + 1) * NPC].reshape(B, NBLK, BLK, T, CIN)
        x38 = np.ones((KX, NBLK, B, BLK), dtype=np.float32)
        x38[: 3 * T] = xs.transpose(3, 4, 1, 0, 2).reshape(3 * T, NBLK, B, BLK)
        per_core.append(
            {
                "A": A_sb,
                "x38": np.ascontiguousarray(x38.reshape(KX, NBLK * B * BLK)).astype(bf),
                "wpad_loc": np.ascontiguousarray(
                    wpad[k * NPC : (k + 1) * NPC]
                    .reshape(NBLK, BLK, L)
                    .transpose(1, 0, 2)
                    .reshape(BLK, NBLK * L)
                ),
            }
        )
    return per_core, wpad_pm, rhs38.astype(bf), L


_KERNEL_CACHE: dict = {}


def _build_kernel(L: int):
    if L in _KERNEL_CACHE:
        return _KERNEL_CACHE[L]

    nc = bacc.Bacc()
    x38_d = nc.declare_dram_parameter("x38", [KX, NBLK * B * BLK], b16, isOutput=False)
    A_d = nc.declare_dram_parameter("A", [128, NBLK * NT * BLK], b16, isOutput=False)
    wpad_d = nc.declare_dram_parameter("wpad", [128, NT * L], f32, isOutput=False)
    wploc_d = nc.declare_dram_parameter("wpad_loc", [BLK, NBLK * L], f32, isOutput=False)
    embT_d = nc.declare_dram_parameter("embT", [D, N], b16, isOutput=False)
    wg_d = nc.declare_dram_parameter("W_gcn", [D, D], b16, isOutput=False)
    bg_d = nc.declare_dram_parameter("b_gcn", [1, D], f32, isOutput=False)
    rhs38_d = nc.declare_dram_parameter("rhs38", [KX, T * D], b16, isOutput=False)
    # output rows padded to 128 per block: SBUF->DRAM writes stripe across
    # all 16 SDMA engines only for full-128-partition tiles (125-row tiles
    # fall back to a 5-engine path at ~1/3 the write bandwidth)
    out_d = nc.declare_dram_parameter("out", [B, NBLK * 128, T, D], b16, isOutput=True)

    with TileContext(nc) as tc:
        with tc.tile_pool(name="const", bufs=1) as cp:
            ones_row = cp.tile([1, BLK], f32)
            nc.vector.memset(ones_row[:], 1.0)

            w_all = cp.tile([128, NT * L], f32)
            wl_all = cp.tile([BLK, NBLK * L], f32)
            nc.scalar.dma_start(out=w_all[:], in_=wpad_d[:])
            nc.scalar.dma_start(out=wl_all[:], in_=wploc_d[:])

            wg0 = cp.tile([128, D], b16)
            wg1 = cp.tile([128, D], b16)
            nc.scalar.dma_start(out=wg0[:], in_=wg_d[0:128, :])
            nc.scalar.dma_start(out=wg1[:], in_=wg_d[128:256, :])
            bg_row = cp.tile([1, D], f32)
            nc.scalar.dma_start(out=bg_row[:], in_=bg_d[:])

            rhs38 = cp.tile([KX, T * D], b16)
            nc.scalar.dma_start(out=rhs38[:], in_=rhs38_d[:])
            x38 = cp.tile([KX, NBLK * B * BLK], b16)
            nc.scalar.dma_start(out=x38[:], in_=x38_d[:])

            A_sb = []
            for blk in range(NBLK):
                a = cp.tile([128, NT * BLK], b16, tag=f"A{blk}")
                nc.scalar.dma_start(
                    out=a[:], in_=A_d[:, blk * NT * BLK : (blk + 1) * NT * BLK]
                )
                A_sb.append(a)

            dinv_all = cp.tile([128, NT], f32)
            dinv_loc = cp.tile([BLK, NBLK], f32)
            g_all = cp.tile([128, NT * D], b16)
            b_rep = cp.tile([BLK, D], f32)

            # ---- phase A: degrees -> dinv (batched reduce + rsqrt) ----
            with (
                tc.tile_pool(name="pA", bufs=2) as pA,
                tc.tile_pool(name="ppA", bufs=2, space="PSUM") as ppA,
                tc.tile_pool(name="emb", bufs=1) as ep,
            ):
                eT0 = ep.tile([128, N], b16)
                eT1 = ep.tile([128, N], b16)
                nc.scalar.dma_start(out=eT0[:], in_=embT_d[0:128, :])
                nc.scalar.dma_start(out=eT1[:], in_=embT_d[128:256, :])

                dega = pA.tile([128, NT], f32, tag="dega")
                nc.vector.reduce_sum(
                    out=dega[:],
                    in_=w_all[:].rearrange("p (j l) -> p j l", l=L),
                    axis=mybir.AxisListType.X,
                )
                reca = pA.tile([128, NT], f32, tag="reca")
                nc.vector.reciprocal(reca[:], dega[:])
                nc.scalar.sqrt(dinv_all[:], reca[:])

                degl = pA.tile([BLK, NBLK], f32, tag="degl")
                nc.vector.reduce_sum(
                    out=degl[:],
                    in_=wl_all[:].rearrange("p (j l) -> p j l", l=L),
                    axis=mybir.AxisListType.X,
                )
                recl = pA.tile([BLK, NBLK], f32, tag="recl")
                nc.vector.reciprocal(recl[:], degl[:])
                nc.scalar.sqrt(dinv_loc[:], recl[:])

                # ---- phase B: g = dinv * (emb @ W_gcn), bf16 in SBUF ----
                nc.vector.memset(g_all[:, (NT - 1) * D :], 0.0)
                for j in range(NT):
                    cols = min(128, N - j * 128)
                    hg = ppA.tile([128, D], f32, space="PSUM", tag="hg")
                    nc.tensor.matmul(
                        hg[:cols, :],
                        lhsT=eT0[:, j * 128 : j * 128 + cols],
                        rhs=wg0[:],
                        start=True,
                        stop=False,
                    )
                    nc.tensor.matmul(
                        hg[:cols, :],
                        lhsT=eT1[:, j * 128 : j * 128 + cols],
                        rhs=wg1[:],
                        start=False,
                        stop=True,
                    )
                    nc.scalar.mul(
                        g_all[:cols, j * D : (j + 1) * D],
                        hg[:cols, :],
                        dinv_all[:cols, j : j + 1],
                    )
                # b_rep = ones(125,1) @ b_gcn(1,256)
                br = ppA.tile([BLK, D], f32, space="PSUM", tag="hg")
                nc.tensor.matmul(
                    br[:], lhsT=ones_row[0:1, :], rhs=bg_row[0:1, :],
                    start=True, stop=True,
                )
                nc.vector.tensor_copy(b_rep[:], br[:])

            # ---- phase C: per block, dense-A GCN matmul then main output ----
            with (
                tc.tile_pool(name="vef", bufs=2) as vef,
                tc.tile_pool(name="veb", bufs=2) as veb,
                tc.tile_pool(name="vps", bufs=2, space="PSUM") as vps,
                tc.tile_pool(name="mps", bufs=2, space="PSUM") as mps,
                tc.tile_pool(name="outp", bufs=3) as outp,
            ):
                for blk in range(NBLK):
                    # vp[dst, :] = sum_src A[src, dst] * g[src, :]
                    vp = vps.tile([BLK, D], f32, space="PSUM", tag="vp")
                    for j in range(NT):
                        nc.tensor.matmul(
                            vp[:],
                            lhsT=A_sb[blk][:, j * BLK : (j + 1) * BLK],
                            rhs=g_all[:, j * D : (j + 1) * D],
                            start=(j == 0),
                            stop=(j == NT - 1),
                        )
                    ve = vef.tile([BLK, D], f32, tag="ve")
                    nc.scalar.mul(ve[:], vp[:], dinv_loc[:, blk : blk + 1])
                    # veps = (ve + b_gcn) tiled over the 6 t-slots of a half
                    veps = veb.tile([BLK, HALF], b16, tag="veps")
                    nc.vector.tensor_tensor(
                        out=veps[:].rearrange("p (t d) -> p t d", d=D),
                        in0=ve[:].rearrange("p d -> p () d").to_broadcast(
                            [BLK, HALF // D, D]
                        ),
                        in1=b_rep[:].rearrange("p d -> p () d").to_broadcast(
                            [BLK, HALF // D, D]
                        ),
                        op=mybir.AluOpType.add,
                    )

                    for b in range(B):
                        lhsT = x38[:, (blk * B + b) * BLK : (blk * B + b + 1) * BLK]
                        osb = outp.tile([128, T * D], b16, tag="osb")
                        for half in range(2):
                            mp = mps.tile([BLK, HALF], f32, space="PSUM", tag="mp")
                            for i in range(3):
                                tp = half * 3 + i
                                nc.tensor.matmul(
                                    mp[:, i * 512 : (i + 1) * 512],
                                    lhsT=lhsT,
                                    rhs=rhs38[:, tp * 512 : (tp + 1) * 512],
                                    start=True,
                                    stop=True,
                                )
                            dst = osb[:BLK, half * HALF : (half + 1) * HALF]
                            h = (blk * B + b) * 2 + half
                            if h % 4 == 0:
                                # route a: DVE adds ve while evacuating PSUM
                                nc.vector.tensor_tensor(
                                    out=dst.rearrange("p (t d) -> p t d", d=D),
                                    in0=mp[:].rearrange("p (t d) -> p t d", d=D),
                                    in1=veps[:].rearrange("p (t d) -> p t d", d=D),
                                    op=mybir.AluOpType.add,
                                )
                            else:
                                # route b: ACT evacuates PSUM to bf16, DVE
                                # adds ve in-place at 2x (16-bit) rate
                                nc.scalar.copy(dst, mp[:])
                                nc.vector.tensor_tensor(
                                    out=dst,
                                    in0=dst,
                                    in1=veps[:],
                                    op=mybir.AluOpType.add,
                                )
                        nc.sync.dma_start(
                            out=out_d[b, blk * 128 : (blk + 1) * 128].rearrange(
                                "p t d -> p (t d)"
                            ),
                            in_=osb[:],
                        )

    nc.finalize()  # run bacc passes (reg alloc, TRN2 sync-wait splitting)
    _KERNEL_CACHE[L] = nc
    return nc


LAST_RESULTS = None  # BassKernelResults of the most recent run (for profiling)


def kernel(x, x_mark, edge_index, weights, W_lin, b_lin, emb_table, W_gcn, b_gcn):
    global LAST_RESULTS
    per_core, wpad, rhs38, L = _prep(x, edge_index, weights, W_lin, b_lin)
    nc = _build_kernel(L)
    embT = np.ascontiguousarray(np.asarray(emb_table, dtype=np.float32).T.astype(bf))
    shared = {
        "wpad": wpad,
        "embT": embT,
        "W_gcn": np.asarray(W_gcn, dtype=np.float32).astype(bf),
        "b_gcn": np.asarray(b_gcn, dtype=np.float32).reshape(1, D),
        "rhs38": rhs38,
    }
    in_maps = [{**shared, **pc} for pc in per_core]
    res = run_bass_kernel_spmd(nc, in_maps, list(range(NCORES)))
    LAST_RESULTS = res
    shards = [
        np.asarray(res.results[k]["out"])
        .reshape(B, NBLK, 128, T, D)[:, :, :BLK]
        .reshape(B, NPC, T, D)
        for k in range(NCORES)
    ]
    return np.concatenate(shards, axis=1).astype(np.float32)


# revision 20
# speedup vs baseline: 1.4617x; 1.0031x over previous
"""Trainium2 Bass kernel for nn_DataEmbedding (linear embed + positional + GCN).

out[b,n,t,:] = x[b,n,t,:] @ W_lin + b_lin + pe[t,:] + gcn(emb_table)[n,:]

Sharding: graph-partitioned by destination node. Core k owns nodes
[625k, 625(k+1)) and produces the output shard out[:, 625k:625(k+1), :, :].
No collectives. Host does index/layout prep only (edge scatter into a dense
adjacency, padding, bf16 casts); all floating-point math runs on device.

GCN message passing is a dense matmul: the host scatters raw edge weights
into A[src, dst] (bf16, [5120 x 625] per core, ~0.6% dense), and the device
computes vp = A^T @ g with 40 accumulating 128-contraction matmuls per
125-node block, where g = D^-1/2 (emb @ W_gcn). Destination normalization
and bias fold into the per-block ve finalize. This replaces the per-edge
indirect-DMA gather + one-hot scatter (which was descriptor-bound).

Main output: per (block, batch) a [38 x 125] bf16 lhsT (x rows + ones rows
for pe/bias) hits a [38 x 3072] rhs; PSUM is evacuated with the ve add,
split between the Vector and Scalar engines, and written to DRAM in bf16.
"""

import numpy as np
import ml_dtypes

import concourse.bacc as bacc
import concourse.bass as bass
import concourse.mybir as mybir
from concourse.bass_utils import run_bass_kernel_spmd
from concourse.tile import TileContext

# problem constants (hardcoded per contract)
B, N, T, CIN, D, E = 8, 5000, 12, 3, 256, 160000
NCORES = 8
NPC = N // NCORES        # nodes per core = 625
BLK = 125                # destination nodes per PSUM block
NBLK = NPC // BLK        # blocks per core = 5
NT = (N + 127) // 128    # global 128-node source tiles = 40
NP = NT * 128            # padded source count = 5120
KX = 3 * T + 2           # main matmul contraction: (t,c) rows + pe + bias = 38
TP = T * D // 512        # 512-col tiles across (t,d) = 6
HALF = 3 * 512           # free elems per evacuation half = 1536

f32 = mybir.dt.float32
b16 = mybir.dt.bfloat16

bf = ml_dtypes.bfloat16


def _pe_table() -> np.ndarray:
    pos = np.arange(T, dtype=np.float32)[:, None]
    div = np.exp(np.arange(0, D, 2, dtype=np.float32) * (-np.log(10000.0) / D))
    pe = np.zeros((T, D), dtype=np.float32)
    pe[:, 0::2] = np.sin(pos * div)
    pe[:, 1::2] = np.cos(pos * div)
    return pe


def _prep(x, edge_index, weights, W_lin, b_lin):
    """Host-side sharding/layout prep: edge scatter, padding, bf16 casts."""
    ei = np.asarray(edge_index)
    row2 = np.concatenate([ei[0], np.arange(N)]).astype(np.int64)  # src
    col2 = np.concatenate([ei[1], np.arange(N)]).astype(np.int64)  # dst
    w2 = np.concatenate(
        [np.asarray(weights, dtype=np.float32), np.ones(N, dtype=np.float32)]
    )

    # dense adjacency A[src, dst] of raw weights (self-loops w=1 included);
    # duplicate (src,dst) edges accumulate, matching segment_sum semantics
    A = np.zeros((NP, N), dtype=np.float32)
    np.add.at(A, (row2, col2), w2)

    # padded per-node incoming-weight matrix for on-device degree = row-sum
    order = np.argsort(col2, kind="stable")
    col_s, w_s = col2[order], w2[order]
    starts = np.searchsorted(col_s, np.arange(N)).astype(np.int64)
    cnt = np.bincount(col2, minlength=N)
    L = int(max(8, ((cnt.max() + 7) // 8) * 8))
    wpad = np.zeros((NP, L), dtype=np.float32)
    offs = np.arange(len(col_s), dtype=np.int64) - starts[col_s]
    wpad[col_s, offs] = w_s
    wpad[N:, 0] = 1.0  # pad rows: deg=1 so dinv stays finite
    wpad_pm = np.ascontiguousarray(
        wpad.reshape(NT, 128, L).transpose(1, 0, 2).reshape(128, NT * L)
    )

    # main-matmul rhs [KX, T*D]: rows 3t+c carry W_lin[c] in the t-block of
    # columns, row 36 = positional encoding, row 37 = b_lin tiled
    pe = _pe_table()
    rhs38 = np.zeros((KX, T * D), dtype=np.float32)
    for t in range(T):
        for c in range(CIN):
            rhs38[3 * t + c, t * D : (t + 1) * D] = np.asarray(W_lin, np.float32)[c]
    rhs38[36] = pe.reshape(-1)
    rhs38[37] = np.tile(np.asarray(b_lin, dtype=np.float32), T)

    xa = np.asarray(x, dtype=np.float32)
    per_core = []
    for k in range(NCORES):
        # A tiles in matmul lhsT layout: [128 src-partition,
        # (blk*NT + j)*BLK + dst-local] bf16
        Ak = A[:, k * NPC : (k + 1) * NPC]
        A_sb = np.ascontiguousarray(
            Ak.reshape(NT, 128, NBLK, BLK)
            .transpose(1, 2, 0, 3)
            .reshape(128, NBLK * NT * BLK)
            .astype(bf)
        )
        # x in matmul-ready lhsT layout [KX, NBLK*B*BLK] bf16: rows are
        # (t,c) pairs then two ones-rows (pe, bias)
        xs = xa[:, k * NPC : (k + 1) * NPC].reshape(B, NBLK, BLK, T, CIN)
        x38 = np.ones((KX, NBLK, B, BLK), dtype=np.float32)
        x38[: 3 * T] = xs.transpose(3, 4, 1, 0, 2).reshape(3 * T, NBLK, B, BLK)
        per_core.append(
            {
                "A": A_sb,
                "x38": np.ascontiguousarray(x38.reshape(KX, NBLK * B * BLK)).astype(bf),
                "wpad_loc": np.ascontiguousarray(
                    wpad[k * NPC : (k + 1) * NPC]
                    .reshape(NBLK, BLK, L)
                    .transpose(1, 0, 2)
                    .reshape(BLK, NBLK * L)
                ),
            }
        )
    return per_core, wpad_pm, rhs38.astype(bf), L


_KERNEL_CACHE: dict = {}


def _build_kernel(L: int):
    if L in _KERNEL_CACHE:
        return _KERNEL_CACHE[L]

    nc = bacc.Bacc()
    x38_d = nc.declare_dram_parameter("x38", [KX, NBLK * B * BLK], b16, isOutput=False)
    A_d = nc.declare_dram_parameter("A", [128, NBLK * NT * BLK], b16, isOutput=False)
    wpad_d = nc.declare_dram_parameter("wpad", [128, NT * L], f32, isOutput=False)
    wploc_d = nc.declare_dram_parameter("wpad_loc", [BLK, NBLK * L], f32, isOutput=False)
    embT_d = nc.declare_dram_parameter("embT", [D, N], b16, isOutput=False)
    wg_d = nc.declare_dram_parameter("W_gcn", [D, D], b16, isOutput=False)
    bg_d = nc.declare_dram_parameter("b_gcn", [1, D], f32, isOutput=False)
    rhs38_d = nc.declare_dram_parameter("rhs38", [KX, T * D], b16, isOutput=False)
    # output rows padded to 128 per block: SBUF->DRAM writes stripe across
    # all 16 SDMA engines only for full-128-partition tiles (125-row tiles
    # fall back to a 5-engine path at ~1/3 the write bandwidth)
    out_d = nc.declare_dram_parameter("out", [B, NBLK * 128, T, D], b16, isOutput=True)

    with TileContext(nc) as tc:
        with tc.tile_pool(name="const", bufs=1) as cp:
            ones_row = cp.tile([1, BLK], f32)
            nc.vector.memset(ones_row[:], 1.0)

            # load order tuned for the critical path: W_gcn + first halves of
            # the embedding table gate phase B; wpad gates dinv; A gates GCN
            wg0 = cp.tile([128, D], b16)
            wg1 = cp.tile([128, D], b16)
            nc.scalar.dma_start(out=wg0[:], in_=wg_d[0:128, :])
            nc.scalar.dma_start(out=wg1[:], in_=wg_d[128:256, :])
            bg_row = cp.tile([1, D], f32)
            nc.scalar.dma_start(out=bg_row[:], in_=bg_d[:])
            rhs38 = cp.tile([KX, T * D], b16)
            nc.scalar.dma_start(out=rhs38[:], in_=rhs38_d[:])

            NSPL = 20 * 128  # embT column split: first 20 source tiles
            eT = [
                [
                    cp.tile([128, NSPL if p == 0 else N - NSPL], b16,
                            name=f"eT{h}{p}", tag=f"eT{h}{p}")
                    for p in range(2)
                ]
                for h in range(2)
            ]
            # both row-halves of the first 20 tiles land first (gate phase B)
            nc.scalar.dma_start(out=eT[0][0][:], in_=embT_d[0:128, :NSPL])
            nc.scalar.dma_start(out=eT[1][0][:], in_=embT_d[128:256, :NSPL])
            w_all = cp.tile([128, NT * L], f32)
            wl_all = cp.tile([BLK, NBLK * L], f32)
            nc.scalar.dma_start(out=w_all[:], in_=wpad_d[:])
            nc.scalar.dma_start(out=wl_all[:], in_=wploc_d[:])
            nc.scalar.dma_start(out=eT[0][1][:], in_=embT_d[0:128, NSPL:])
            nc.scalar.dma_start(out=eT[1][1][:], in_=embT_d[128:256, NSPL:])

            x38 = cp.tile([KX, NBLK * B * BLK], b16)
            nc.scalar.dma_start(out=x38[:], in_=x38_d[:])

            A_sb = []
            for blk in range(NBLK):
                a = cp.tile([128, NT * BLK], b16, tag=f"A{blk}")
                nc.scalar.dma_start(
                    out=a[:], in_=A_d[:, blk * NT * BLK : (blk + 1) * NT * BLK]
                )
                A_sb.append(a)

            dinv_all = cp.tile([128, NT], f32)
            dinv_loc = cp.tile([BLK, NBLK], f32)
            g_all = cp.tile([128, NT * D], b16)
            b_rep = cp.tile([BLK, D], f32)

            # ---- phase A: degrees -> dinv (batched reduce + rsqrt) ----
            with (
                tc.tile_pool(name="pA", bufs=2) as pA,
                tc.tile_pool(name="ppA", bufs=2, space="PSUM") as ppA,
            ):
                dega = pA.tile([128, NT], f32, tag="dega")
                nc.vector.reduce_sum(
                    out=dega[:],
                    in_=w_all[:].rearrange("p (j l) -> p j l", l=L),
                    axis=mybir.AxisListType.X,
                )
                reca = pA.tile([128, NT], f32, tag="reca")
                nc.vector.reciprocal(reca[:], dega[:])
                nc.scalar.sqrt(dinv_all[:], reca[:])

                degl = pA.tile([BLK, NBLK], f32, tag="degl")
                nc.vector.reduce_sum(
                    out=degl[:],
                    in_=wl_all[:].rearrange("p (j l) -> p j l", l=L),
                    axis=mybir.AxisListType.X,
                )
                recl = pA.tile([BLK, NBLK], f32, tag="recl")
                nc.vector.reciprocal(recl[:], degl[:])
                nc.scalar.sqrt(dinv_loc[:], recl[:])

                # ---- phase B: g = dinv * (emb @ W_gcn), bf16 in SBUF ----
                nc.vector.memset(g_all[:, (NT - 1) * D :], 0.0)
                for j in range(NT):
                    part = 0 if j < 20 else 1
                    jc = (j - 20 * part) * 128
                    cols = min(128, N - j * 128)
                    hg = ppA.tile([128, D], f32, space="PSUM", tag="hg")
                    nc.tensor.matmul(
                        hg[:cols, :],
                        lhsT=eT[0][part][:, jc : jc + cols],
                        rhs=wg0[:],
                        start=True,
                        stop=False,
                    )
                    nc.tensor.matmul(
                        hg[:cols, :],
                        lhsT=eT[1][part][:, jc : jc + cols],
                        rhs=wg1[:],
                        start=False,
                        stop=True,
                    )
                    nc.scalar.mul(
                        g_all[:cols, j * D : (j + 1) * D],
                        hg[:cols, :],
                        dinv_all[:cols, j : j + 1],
                    )
                # b_rep = ones(125,1) @ b_gcn(1,256)
                br = ppA.tile([BLK, D], f32, space="PSUM", tag="hg")
                nc.tensor.matmul(
                    br[:], lhsT=ones_row[0:1, :], rhs=bg_row[0:1, :],
                    start=True, stop=True,
                )
                nc.vector.tensor_copy(b_rep[:], br[:])

            # ---- phase C: per block, dense-A GCN matmul then main output ----
            with (
                tc.tile_pool(name="vef", bufs=2) as vef,
                tc.tile_pool(name="veb", bufs=2) as veb,
                tc.tile_pool(name="vps", bufs=2, space="PSUM") as vps,
                tc.tile_pool(name="mps", bufs=2, space="PSUM") as mps,
                tc.tile_pool(name="outp", bufs=3) as outp,
            ):
                # vp[dst, :] = sum_src A[src, dst] * g[src, :]; block blk+1's
                # accumulation chunks are interleaved into block blk's main
                # matmuls so the PE never drains (keeps the p-state ramped)
                vp_tiles: dict = {}

                def gcn_chunk(blk, j0, j1):
                    if blk not in vp_tiles:
                        vp_tiles[blk] = vps.tile(
                            [BLK, D], f32, space="PSUM", name="vp", tag="vp"
                        )
                    for j in range(j0, j1):
                        nc.tensor.matmul(
                            vp_tiles[blk][:],
                            lhsT=A_sb[blk][:, j * BLK : (j + 1) * BLK],
                            rhs=g_all[:, j * D : (j + 1) * D],
                            start=(j == 0),
                            stop=(j == NT - 1),
                            skip_group_check=True,
                        )

                gcn_chunk(0, 0, NT)
                for blk in range(NBLK):
                    vp = vp_tiles.pop(blk)
                    ve = vef.tile([BLK, D], f32, tag="ve")
                    nc.scalar.mul(ve[:], vp[:], dinv_loc[:, blk : blk + 1])
                    # veps = (ve + b_gcn) tiled over the 6 t-slots of a half
                    veps = veb.tile([BLK, HALF], b16, tag="veps")
                    nc.vector.tensor_tensor(
                        out=veps[:].rearrange("p (t d) -> p t d", d=D),
                        in0=ve[:].rearrange("p d -> p () d").to_broadcast(
                            [BLK, HALF // D, D]
                        ),
                        in1=b_rep[:].rearrange("p d -> p () d").to_broadcast(
                            [BLK, HALF // D, D]
                        ),
                        op=mybir.AluOpType.add,
                    )

                    for b in range(B):
                        lhsT = x38[:, (blk * B + b) * BLK : (blk * B + b + 1) * BLK]
                        osb = outp.tile([128, T * D], b16, tag="osb")
                        for half in range(2):
                            mp = mps.tile([BLK, HALF], f32, space="PSUM", tag="mp")
                            for i in range(3):
                                tp = half * 3 + i
                                nc.tensor.matmul(
                                    mp[:, i * 512 : (i + 1) * 512],
                                    lhsT=lhsT,
                                    rhs=rhs38[:, tp * 512 : (tp + 1) * 512],
                                    start=True,
                                    stop=True,
                                )
                            dst = osb[:BLK, half * HALF : (half + 1) * HALF]
                            h = (blk * B + b) * 2 + half
                            if h % 4 == 0 or h % 16 == 1:
                                # route a: DVE adds ve while evacuating PSUM
                                nc.vector.tensor_tensor(
                                    out=dst.rearrange("p (t d) -> p t d", d=D),
                                    in0=mp[:].rearrange("p (t d) -> p t d", d=D),
                                    in1=veps[:].rearrange("p (t d) -> p t d", d=D),
                                    op=mybir.AluOpType.add,
                                )
                            else:
                                # route b: ACT evacuates PSUM to bf16, DVE
                                # adds ve in-place at 2x (16-bit) rate
                                nc.scalar.copy(dst, mp[:])
                                nc.vector.tensor_tensor(
                                    out=dst,
                                    in0=dst,
                                    in1=veps[:],
                                    op=mybir.AluOpType.add,
                                )
                        nc.sync.dma_start(
                            out=out_d[b, blk * 128 : (blk + 1) * 128].rearrange(
                                "p t d -> p (t d)"
                            ),
                            in_=osb[:],
                        )
                        if blk + 1 < NBLK and b in (2, 5):
                            gcn_chunk(blk + 1, 0 if b == 2 else 20, 20 if b == 2 else NT)

    nc.finalize()  # run bacc passes (reg alloc, TRN2 sync-wait splitting)
    _KERNEL_CACHE[L] = nc
    return nc


LAST_RESULTS = None  # BassKernelResults of the most recent run (for profiling)


def kernel(x, x_mark, edge_index, weights, W_lin, b_lin, emb_table, W_gcn, b_gcn):
    global LAST_RESULTS
    per_core, wpad, rhs38, L = _prep(x, edge_index, weights, W_lin, b_lin)
    nc = _build_kernel(L)
    embT = np.ascontiguousarray(np.asarray(emb_table, dtype=np.float32).T.astype(bf))
    shared = {
        "wpad": wpad,
        "embT": embT,
        "W_gcn": np.asarray(W_gcn, dtype=np.float32).astype(bf),
        "b_gcn": np.asarray(b_gcn, dtype=np.float32).reshape(1, D),
        "rhs38": rhs38,
    }
    in_maps = [{**shared, **pc} for pc in per_core]
    res = run_bass_kernel_spmd(nc, in_maps, list(range(NCORES)))
    LAST_RESULTS = res
    shards = [
        np.asarray(res.results[k]["out"])
        .reshape(B, NBLK, 128, T, D)[:, :, :BLK]
        .reshape(B, NPC, T, D)
        for k in range(NCORES)
    ]
    return np.concatenate(shards, axis=1).astype(np.float32)


# revision 29
# speedup vs baseline: 1.4988x; 1.0254x over previous
"""Trainium2 Bass kernel for nn_DataEmbedding (linear embed + positional + GCN).

out[b,n,t,:] = x[b,n,t,:] @ W_lin + b_lin + pe[t,:] + gcn(emb_table)[n,:]

Sharding: graph-partitioned by destination node. Core k owns nodes
[625k, 625(k+1)) and produces the output shard out[:, 625k:625(k+1), :, :].
No collectives. Host does index/layout prep only (edge scatter into a dense
adjacency, padding, bf16 casts); all floating-point math runs on device.

GCN message passing is a dense matmul: the host scatters raw edge weights
into A[src, dst] (bf16, [5120 x 625] per core, ~0.6% dense), and the device
computes vp = A^T @ g with 40 accumulating 128-contraction matmuls per
125-node block, where g = D^-1/2 (emb @ W_gcn). Destination normalization
and bias fold into the per-block ve finalize. This replaces the per-edge
indirect-DMA gather + one-hot scatter (which was descriptor-bound).

Main output: per (block, batch) a [38 x 125] bf16 lhsT (x rows + ones rows
for pe/bias) hits a [38 x 3072] rhs; PSUM is evacuated with the ve add,
split between the Vector and Scalar engines, and written to DRAM in bf16.
"""

import numpy as np
import ml_dtypes

import concourse.bacc as bacc
import concourse.bass as bass
import concourse.mybir as mybir
from concourse.bass_utils import run_bass_kernel_spmd
from concourse.tile import TileContext

# problem constants (hardcoded per contract)
B, N, T, CIN, D, E = 8, 5000, 12, 3, 256, 160000
NCORES = 8
NPC = N // NCORES        # nodes per core = 625
BLK = 125                # destination nodes per PSUM block
NBLK = NPC // BLK        # blocks per core = 5
NT = (N + 127) // 128    # global 128-node source tiles = 40
NP = NT * 128            # padded source count = 5120
KX = 3 * T + 2           # main matmul contraction: (t,c) rows + pe + bias = 38
TP = T * D // 512        # 512-col tiles across (t,d) = 6
HALF = 3 * 512           # free elems per evacuation half = 1536

f32 = mybir.dt.float32
b16 = mybir.dt.bfloat16

bf = ml_dtypes.bfloat16


def _pe_table() -> np.ndarray:
    pos = np.arange(T, dtype=np.float32)[:, None]
    div = np.exp(np.arange(0, D, 2, dtype=np.float32) * (-np.log(10000.0) / D))
    pe = np.zeros((T, D), dtype=np.float32)
    pe[:, 0::2] = np.sin(pos * div)
    pe[:, 1::2] = np.cos(pos * div)
    return pe


def _prep(x, edge_index, weights, W_lin, b_lin, b_gcn):
    """Host-side sharding/layout prep: edge scatter, padding, bf16 casts."""
    ei = np.asarray(edge_index)
    row2 = np.concatenate([ei[0], np.arange(N)]).astype(np.int64)  # src
    col2 = np.concatenate([ei[1], np.arange(N)]).astype(np.int64)  # dst
    w2 = np.concatenate(
        [np.asarray(weights, dtype=np.float32), np.ones(N, dtype=np.float32)]
    )

    # dense adjacency A[src, dst] of raw weights (self-loops w=1 included);
    # duplicate (src,dst) edges accumulate, matching segment_sum semantics
    A = np.zeros((NP, N), dtype=np.float32)
    np.add.at(A, (row2, col2), w2)

    # padded per-node incoming-weight matrix for on-device degree = row-sum
    order = np.argsort(col2, kind="stable")
    col_s, w_s = col2[order], w2[order]
    starts = np.searchsorted(col_s, np.arange(N)).astype(np.int64)
    cnt = np.bincount(col2, minlength=N)
    L = int(max(8, ((cnt.max() + 7) // 8) * 8))
    wpad = np.zeros((NP, L), dtype=np.float32)
    offs = np.arange(len(col_s), dtype=np.int64) - starts[col_s]
    wpad[col_s, offs] = w_s
    wpad[N:, 0] = 1.0  # pad rows: deg=1 so dinv stays finite
    wpad_pm = np.ascontiguousarray(
        wpad.reshape(NT, 128, L).transpose(1, 0, 2).reshape(128, NT * L)
    )

    # main-matmul rhs [KX, T*D]: rows 3t+c carry W_lin[c] in the t-block of
    # columns, row 36 = positional encoding, row 37 = b_lin tiled
    pe = _pe_table()
    rhs38 = np.zeros((KX, T * D), dtype=np.float32)
    for t in range(T):
        for c in range(CIN):
            rhs38[3 * t + c, t * D : (t + 1) * D] = np.asarray(W_lin, np.float32)[c]
    rhs38[36] = pe.reshape(-1)
    # row 37 multiplies a ones-row: carries both biases (b_lin and the GCN
    # bias, which is added uniformly over (n, t))
    rhs38[37] = np.tile(
        np.asarray(b_lin, dtype=np.float32) + np.asarray(b_gcn, dtype=np.float32), T
    )

    xa = np.asarray(x, dtype=np.float32)
    per_core = []
    for k in range(NCORES):
        # A tiles in matmul lhsT layout: [128 src-partition,
        # (blk*NT + j)*BLK + dst-local] bf16
        Ak = A[:, k * NPC : (k + 1) * NPC]
        A_sb = np.ascontiguousarray(
            Ak.reshape(NT, 128, NBLK, BLK)
            .transpose(1, 2, 0, 3)
            .reshape(128, NBLK * NT * BLK)
            .astype(bf)
        )
        # x in matmul-ready lhsT layout [KX, NBLK*B*BLK] bf16: rows are
        # (t,c) pairs then two ones-rows (pe, bias)
        xs = xa[:, k * NPC : (k + 1) * NPC].reshape(B, NBLK, BLK, T, CIN)
        x38 = np.ones((KX, NBLK, B, BLK), dtype=np.float32)
        x38[: 3 * T] = xs.transpose(3, 4, 1, 0, 2).reshape(3 * T, NBLK, B, BLK)
        per_core.append(
            {
                "A": A_sb,
                "x38": np.ascontiguousarray(x38.reshape(KX, NBLK * B * BLK)).astype(bf),
                "wpad_loc": np.ascontiguousarray(
                    wpad[k * NPC : (k + 1) * NPC]
                    .reshape(NBLK, BLK, L)
                    .transpose(1, 0, 2)
                    .reshape(BLK, NBLK * L)
                ),
            }
        )
    return per_core, wpad_pm, rhs38.astype(bf), L


_KERNEL_CACHE: dict = {}


def _build_kernel(L: int):
    if L in _KERNEL_CACHE:
        return _KERNEL_CACHE[L]

    nc = bacc.Bacc()
    x38_d = nc.declare_dram_parameter("x38", [KX, NBLK * B * BLK], b16, isOutput=False)
    A_d = nc.declare_dram_parameter("A", [128, NBLK * NT * BLK], b16, isOutput=False)
    wpad_d = nc.declare_dram_parameter("wpad", [128, NT * L], f32, isOutput=False)
    wploc_d = nc.declare_dram_parameter("wpad_loc", [BLK, NBLK * L], f32, isOutput=False)
    embT_d = nc.declare_dram_parameter("embT", [D, N], b16, isOutput=False)
    wg_d = nc.declare_dram_parameter("W_gcn", [D, D], b16, isOutput=False)
    rhs38_d = nc.declare_dram_parameter("rhs38", [KX, T * D], b16, isOutput=False)
    # output rows padded to 128 per block: SBUF->DRAM writes stripe across
    # all 16 SDMA engines only for full-128-partition tiles (125-row tiles
    # fall back to a 5-engine path at ~1/3 the write bandwidth)
    out_d = nc.declare_dram_parameter("out", [B, NBLK * 128, T, D], b16, isOutput=True)

    with TileContext(nc) as tc:
        with tc.tile_pool(name="const", bufs=1) as cp:
            ones_row = cp.tile([1, BLK], f32)
            nc.vector.memset(ones_row[:], 1.0)
            # warm the Sqrt activation table (~2.6us load) during the DMAs so
            # the dinv sqrt on the critical path doesn't pay it
            warmup = cp.tile([1, BLK], f32)
            nc.scalar.sqrt(warmup[:], ones_row[:])

            # load order tuned for the critical path: wpad gates dinv which
            # gates phase B's PSUM evacuation; W_gcn + first halves of the
            # embedding table gate phase B matmuls; A gates the block GCN
            wg0 = cp.tile([128, D], b16)
            wg1 = cp.tile([128, D], b16)
            nc.scalar.dma_start(out=wg0[:], in_=wg_d[0:128, :])
            nc.scalar.dma_start(out=wg1[:], in_=wg_d[128:256, :])
            rhs38 = cp.tile([KX, T * D], b16)
            nc.scalar.dma_start(out=rhs38[:], in_=rhs38_d[:])
            w_all = cp.tile([128, NT * L], f32)
            wl_all = cp.tile([BLK, NBLK * L], f32)
            nc.scalar.dma_start(out=w_all[:], in_=wpad_d[:])
            nc.scalar.dma_start(out=wl_all[:], in_=wploc_d[:])

            NSPL = 20 * 128  # embT column split: first 20 source tiles
            eT = [
                [
                    cp.tile([128, NSPL if p == 0 else N - NSPL], b16,
                            name=f"eT{h}{p}", tag=f"eT{h}{p}")
                    for p in range(2)
                ]
                for h in range(2)
            ]
            # both row-halves of the first 20 tiles land first (gate phase B)
            nc.scalar.dma_start(out=eT[0][0][:], in_=embT_d[0:128, :NSPL])
            nc.scalar.dma_start(out=eT[1][0][:], in_=embT_d[128:256, :NSPL])
            nc.scalar.dma_start(out=eT[0][1][:], in_=embT_d[0:128, NSPL:])
            nc.scalar.dma_start(out=eT[1][1][:], in_=embT_d[128:256, NSPL:])

            x38 = cp.tile([KX, NBLK * B * BLK], b16)
            nc.scalar.dma_start(out=x38[:], in_=x38_d[:])

            A_sb = []
            for blk in range(NBLK):
                a = cp.tile([128, NT * BLK], b16, tag=f"A{blk}")
                nc.scalar.dma_start(
                    out=a[:], in_=A_d[:, blk * NT * BLK : (blk + 1) * NT * BLK]
                )
                A_sb.append(a)

            dinv_all = cp.tile([128, NT], f32)
            dinv_loc = cp.tile([BLK, NBLK], f32)
            g_all = cp.tile([128, NT * D], b16)

            # ---- phase A: degrees -> dinv (batched reduce + rsqrt) ----
            with (
                tc.tile_pool(name="pA", bufs=2) as pA,
                tc.tile_pool(name="ppA", bufs=2, space="PSUM") as ppA,
            ):
                dega = pA.tile([128, NT], f32, tag="dega")
                nc.vector.reduce_sum(
                    out=dega[:],
                    in_=w_all[:].rearrange("p (j l) -> p j l", l=L),
                    axis=mybir.AxisListType.X,
                )
                reca = pA.tile([128, NT], f32, tag="reca")
                nc.vector.reciprocal(reca[:], dega[:])
                nc.scalar.sqrt(dinv_all[:], reca[:])

                degl = pA.tile([BLK, NBLK], f32, tag="degl")
                nc.vector.reduce_sum(
                    out=degl[:],
                    in_=wl_all[:].rearrange("p (j l) -> p j l", l=L),
                    axis=mybir.AxisListType.X,
                )
                recl = pA.tile([BLK, NBLK], f32, tag="recl")
                nc.vector.reciprocal(recl[:], degl[:])
                nc.scalar.sqrt(dinv_loc[:], recl[:])

                # ---- phase B: g = dinv * (emb @ W_gcn), bf16 in SBUF ----
                nc.vector.memset(g_all[:, (NT - 1) * D :], 0.0)
                for j in range(NT):
                    part = 0 if j < 20 else 1
                    jc = (j - 20 * part) * 128
                    cols = min(128, N - j * 128)
                    hg = ppA.tile([128, D], f32, space="PSUM", tag="hg")
                    nc.tensor.matmul(
                        hg[:cols, :],
                        lhsT=eT[0][part][:, jc : jc + cols],
                        rhs=wg0[:],
                        start=True,
                        stop=False,
                    )
                    nc.tensor.matmul(
                        hg[:cols, :],
                        lhsT=eT[1][part][:, jc : jc + cols],
                        rhs=wg1[:],
                        start=False,
                        stop=True,
                    )
                    # alternate the scale between ACT and DVE so neither
                    # engine walls phase B's PSUM recycling
                    if j % 2 == 0:
                        nc.scalar.mul(
                            g_all[:cols, j * D : (j + 1) * D],
                            hg[:cols, :],
                            dinv_all[:cols, j : j + 1],
                        )
                    else:
                        nc.vector.tensor_scalar_mul(
                            g_all[:cols, j * D : (j + 1) * D],
                            hg[:cols, :],
                            dinv_all[:cols, j : j + 1],
                        )

            # ---- phase C: per block, dense-A GCN matmul then main output ----
            with (
                tc.tile_pool(name="vef", bufs=2) as vef,
                tc.tile_pool(name="veb", bufs=2) as veb,
                tc.tile_pool(name="vps", bufs=2, space="PSUM") as vps,
                tc.tile_pool(name="mps", bufs=2, space="PSUM") as mps,
                tc.tile_pool(name="outp", bufs=3) as outp,
            ):
                # vp[dst, :] = sum_src A[src, dst] * g[src, :]; block blk+1's
                # accumulation chunks are interleaved into block blk's main
                # matmuls so the PE never drains (keeps the p-state ramped)
                vp_tiles: dict = {}

                def gcn_chunk(blk, j0, j1):
                    if blk not in vp_tiles:
                        vp_tiles[blk] = vps.tile(
                            [BLK, D], f32, space="PSUM", name="vp", tag="vp"
                        )
                    for j in range(j0, j1):
                        nc.tensor.matmul(
                            vp_tiles[blk][:],
                            lhsT=A_sb[blk][:, j * BLK : (j + 1) * BLK],
                            rhs=g_all[:, j * D : (j + 1) * D],
                            start=(j == 0),
                            stop=(j == NT - 1),
                            skip_group_check=True,
                        )

                gcn_chunk(0, 0, NT)
                for blk in range(NBLK):
                    vp = vp_tiles.pop(blk)
                    ve = vef.tile([BLK, D], b16, tag="ve")
                    nc.scalar.mul(ve[:], vp[:], dinv_loc[:, blk : blk + 1])
                    # veps = ve tiled over all 12 t-slots (b_gcn rides the
                    # main matmul's bias row)
                    veps = veb.tile([BLK, T * D], b16, tag="veps")
                    nc.vector.tensor_copy(
                        veps[:].rearrange("p (t d) -> p t d", d=D),
                        ve[:].rearrange("p d -> p () d").to_broadcast([BLK, T, D]),
                    )

                    for b in range(B):
                        lhsT = x38[:, (blk * B + b) * BLK : (blk * B + b + 1) * BLK]
                        osb = outp.tile([128, T * D], b16, tag="osb")
                        route_a = b in (0, 4)
                        for half in range(2):
                            mp = mps.tile([BLK, HALF], f32, space="PSUM", tag="mp")
                            for i in range(3):
                                tp = half * 3 + i
                                nc.tensor.matmul(
                                    mp[:, i * 512 : (i + 1) * 512],
                                    lhsT=lhsT,
                                    rhs=rhs38[:, tp * 512 : (tp + 1) * 512],
                                    start=True,
                                    stop=True,
                                )
                            dst = osb[:BLK, half * HALF : (half + 1) * HALF]
                            if route_a:
                                # route a: DVE adds ve while evacuating PSUM
                                nc.vector.tensor_tensor(
                                    out=dst.rearrange("p (t d) -> p t d", d=D),
                                    in0=mp[:].rearrange("p (t d) -> p t d", d=D),
                                    in1=veps[:, half * HALF : (half + 1) * HALF]
                                    .rearrange("p (t d) -> p t d", d=D),
                                    op=mybir.AluOpType.add,
                                )
                            else:
                                # route b: ACT evacuates PSUM to bf16 per
                                # half; one DVE in-place add at 2x (16-bit)
                                # rate covers both halves after the second
                                nc.scalar.copy(dst, mp[:])
                        if not route_a:
                            nc.vector.tensor_tensor(
                                out=osb[:BLK, :],
                                in0=osb[:BLK, :],
                                in1=veps[:],
                                op=mybir.AluOpType.add,
                            )
                        nc.sync.dma_start(
                            out=out_d[b, blk * 128 : (blk + 1) * 128].rearrange(
                                "p t d -> p (t d)"
                            ),
                            in_=osb[:],
                        )
                        if blk + 1 < NBLK and b in (2, 5):
                            gcn_chunk(blk + 1, 0 if b == 2 else 20, 20 if b == 2 else NT)

    nc.finalize()  # run bacc passes (reg alloc, TRN2 sync-wait splitting)
    _KERNEL_CACHE[L] = nc
    return nc


LAST_RESULTS = None  # BassKernelResults of the most recent run (for profiling)


def kernel(x, x_mark, edge_index, weights, W_lin, b_lin, emb_table, W_gcn, b_gcn):
    global LAST_RESULTS
    per_core, wpad, rhs38, L = _prep(x, edge_index, weights, W_lin, b_lin, b_gcn)
    nc = _build_kernel(L)
    embT = np.ascontiguousarray(np.asarray(emb_table, dtype=np.float32).T.astype(bf))
    shared = {
        "wpad": wpad,
        "embT": embT,
        "W_gcn": np.asarray(W_gcn, dtype=np.float32).astype(bf),
        "rhs38": rhs38,
    }
    in_maps = [{**shared, **pc} for pc in per_core]
    res = run_bass_kernel_spmd(nc, in_maps, list(range(NCORES)))
    LAST_RESULTS = res
    shards = [
        np.asarray(res.results[k]["out"])
        .reshape(B, NBLK, 128, T, D)[:, :, :BLK]
        .reshape(B, NPC, T, D)
        for k in range(NCORES)
    ]
    return np.concatenate(shards, axis=1).astype(np.float32)
